# revision 21
# baseline (speedup 1.0000x reference)
"""Trainium2 Bass kernel for nn_CapsuleLayer (capsule conv + 3-iter routing).

Reference (per batch image, C=128, H=W=32, K=3, pad=1):
  priors[h,w,t,nc] = sum_c x_pad[c,h+i,w+j] * W[t,c,nc] + b[t,nc]
  o = mean_t priors
  3x: d2 = sum_cch (o - p_t)^2 ; cw = rsqrt(d2 + 1e-4)
      cw = cw / sum_t cw ; o = sum_t cw_t p_t
  out[nc,h,w] = o

Sharding: data-parallel over batch; 8 cores, one image each; weight/bias
replicated; no collectives.

Implementation notes:
- host-side prep: input/weight/bias cast to bf16 and weight permuted to
  (cch,cap) column order with c first, so the device does no weight
  permutes and the weight DMA is one contiguous read.
- xs[j] (column-shifted padded images): xs[1] is one clean DMA of x;
  xs[0]/xs[2] are one-column shifts copied on the idle DVE during the
  prologue.
- bf16 on-chip (fp32 PSUM accumulation in matmuls); rel err ~1e-2 vs the
  fp32 reference, within the 2e-2 gate.
- priors layout [128pos, grp, tap9, cch16, cap32]: innermost step-1 cap
  runs keep every big DVE op in 2x_1P mode; routing processes GRP=4
  position-chunks per pass to amortize per-op overhead (GRP=2 in the
  bias variant, which needs extra SBUF).
- ||o - p||^2 = ||p||^2 - <2o, p> + ||o||^2: per iteration two full-size
  DVE passes (product o2*p, product p*alpha) with their halving-add
  reductions (2x mode) done IN PLACE inside the product tiles (frees the
  former h1/wh staging pool -> GRP=4 fits SBUF).
- e' = ||o||^2 + eps computed as 0.25*sum_cch(o2^2): the square runs on
  ACT straight off o2, so no dependency on s and no alpha*s pass.
- mean_t priors via 9 extra accumulating matmuls on the idle PE.
- rsqrt = Abs_reciprocal_sqrt on ACT: lives in one table set together
  with square/copy/identity -> no ACT table switching.
- output staged bf16, cast to f32 by the SWDGE DMA on the idle gpsimd
  queue.
"""

import numpy as np

C = 128
H = W = 32
B = 8
KK = 9
NCAPS = 32
CCH = 16
NC = NCAPS * CCH  # 512
NIT = 3
NPOS = H * W
CHUNK = 128
ROWS = H + 2  # 34 row-slots in the shifted images

_cache = {}


def _build(with_bias: bool):
    import concourse.bass as bass
    import concourse.tile as tile
    from concourse import bacc, mybir
    from concourse.masks import make_identity

    GRP = 2 if with_bias else 4  # position-chunks per routing pass
    NGRP = NPOS // (CHUNK * GRP)

    f32 = mybir.dt.float32
    bf16 = mybir.dt.bfloat16
    AF = mybir.ActivationFunctionType

    nc = bacc.Bacc()
    x_d = nc.dram_tensor("x", [C, H, W], bf16, kind="ExternalInput")
    w_d = nc.dram_tensor("w", [C, KK, NC], bf16, kind="ExternalInput")
    b_d = nc.dram_tensor("b", [KK, NC], bf16, kind="ExternalInput")
    out_d = nc.dram_tensor("out", [NC, NPOS], f32, kind="ExternalOutput")

    with tile.TileContext(nc) as tc:
        with (
            tc.tile_pool(name="singles", bufs=1) as singles,
            tc.tile_pool(name="priors", bufs=2) as priors_pool,
            tc.tile_pool(name="big", bufs=2) as big_pool,
            tc.tile_pool(name="o", bufs=2) as o_pool,
            tc.tile_pool(name="npool", bufs=1) as npool,
            tc.tile_pool(name="sqp", bufs=1) as sqp,
            tc.tile_pool(name="small", bufs=2) as small_pool,
            tc.tile_pool(name="pp", bufs=4, space="PSUM") as pp,
            tc.tile_pool(name="mp", bufs=2, space="PSUM") as mp,
            tc.tile_pool(name="tpp", bufs=2, space="PSUM") as tpp,
        ):
            # ---- stage inputs ----
            # column-shifted padded images: xs[j][c, r*32+w] = x[c, r-1, w+j-1]
            # (zero out of range) -> the (i,j)-tap patch for rows h0.. is the
            # CONTIGUOUS slice xs[j][:, (h0+i)*32 : (h0+i)*32+128].
            xs = []
            xviews = []
            for j in range(3):
                xj = singles.tile([C, ROWS * W], bf16, tag=f"xs{j}")
                xs.append(xj)
                xviews.append(xj[:].rearrange("p (r w) -> p r w", r=ROWS))
            nc.sync.dma_start(out=xviews[1][:, 1 : H + 1, :], in_=x_d[:])
            for j in range(3):
                nc.gpsimd.memset(xviews[j][:, 0], 0.0)
                nc.gpsimd.memset(xviews[j][:, ROWS - 1], 0.0)
            nc.gpsimd.memset(xviews[0][:, 1 : H + 1, 0], 0.0)
            nc.gpsimd.memset(xviews[2][:, 1 : H + 1, W - 1], 0.0)
            # the shifts run on the DVE, idle during the prologue
            nc.vector.tensor_copy(
                out=xviews[0][:, 1 : H + 1, 1:W],
                in_=xviews[1][:, 1 : H + 1, 0 : W - 1],
            )
            nc.vector.tensor_copy(
                out=xviews[2][:, 1 : H + 1, 0 : W - 1],
                in_=xviews[1][:, 1 : H + 1, 1:W],
            )

            # weights arrive pre-permuted to (cch, cap) column order; one
            # contiguous HWDGE DMA for all 9 taps, issued on the idle ACT
            # queue so it overlaps the x DMA issue on sync
            w_all = singles.tile([C, KK, NC], bf16)
            nc.scalar.dma_start(out=w_all[:], in_=w_d[:])
            wsb = [w_all[:, t] for t in range(KK)]

            ident = singles.tile([128, 128], f32)
            make_identity(nc, ident[:])

            eps = singles.tile([128, 1], f32)
            nc.gpsimd.memset(eps, 1e-4)

            if with_bias:
                bsb = singles.tile([1, KK, NC], bf16)
                nc.gpsimd.dma_start(out=bsb[:], in_=b_d[:].unsqueeze(0))
                ones = singles.tile([1, CHUNK], bf16)
                nc.gpsimd.memset(ones, 1.0)

            def emit_e2(o2):
                # e' = ||o||^2 + eps = 0.25*sum_cch(o2^2) + eps.  The square
                # runs on ACT; only the short cch ladder lands on the DVE.
                # Replaces sum_t alpha_t s_t (same value, no s dependency).
                sq = sqp.tile([128, GRP, CCH, NCAPS], bf16)
                nc.scalar.activation(
                    out=sq[:],
                    in_=o2[:].rearrange("p c (a b) -> p c a b", a=CCH),
                    func=AF.Square,
                )
                nc.vector.tensor_add(sq[:, :, 0:8], sq[:, :, 0:8], sq[:, :, 8:16])
                nc.vector.tensor_add(sq[:, :, 0:4], sq[:, :, 0:4], sq[:, :, 4:8])
                nc.vector.tensor_add(sq[:, :, 0:2], sq[:, :, 0:2], sq[:, :, 2:4])
                e2 = small_pool.tile([128, GRP, NCAPS], f32, tag="e2")
                nc.vector.tensor_add(e2[:], sq[:, :, 0], sq[:, :, 1])
                e2b = small_pool.tile([128, GRP, NCAPS], bf16, tag="e2b")
                nc.scalar.activation(
                    out=e2b[:], in_=e2[:], func=AF.Identity,
                    bias=eps[:], scale=0.25,
                )
                return e2b

            def cch_ladder_inplace(tp):
                # reduce [128, ..., CCH, NCAPS] over cch in place; result in
                # tp[..., 0, :]
                nc.vector.tensor_add(tp[..., 0:8, :], tp[..., 0:8, :], tp[..., 8:16, :])
                nc.vector.tensor_add(tp[..., 0:4, :], tp[..., 0:4, :], tp[..., 4:8, :])
                nc.vector.tensor_add(tp[..., 0:2, :], tp[..., 0:2, :], tp[..., 2:4, :])
                nc.vector.tensor_add(tp[..., 0, :], tp[..., 0, :], tp[..., 1, :])

            for g in range(NGRP):
                # ---- priors + mean via PE ----
                priors = priors_pool.tile([128, GRP, KK, CCH, NCAPS], bf16)
                o2 = o_pool.tile([128, GRP, NC], bf16)
                # it0's s-product, filled per sub-chunk as each chunk's mean
                # lands (prologue fill)
                tprod0 = big_pool.tile([128, GRP, KK, CCH, NCAPS], bf16, tag="big")
                for cc in range(GRP):
                    ch = GRP * g + cc
                    om = mp.tile([128, NC], f32)  # sum_t priors (fp32)
                    # j=1 taps first: xs[1] is a direct DMA and lands before
                    # the shifted xs[0]/xs[2] copies
                    taps = (1, 4, 7, 0, 3, 6, 2, 5, 8)
                    for ti, t in enumerate(taps):
                        i, j = divmod(t, 3)
                        ps = pp.tile([128, NC], f32)
                        lhsT = xs[j][:, 128 * ch + 32 * i : 128 * ch + 32 * i + 128]
                        rhs = wsb[t][:]
                        if with_bias:
                            nc.tensor.matmul(
                                ps[:], lhsT, rhs, start=True, stop=False
                            )
                            brhs = bsb[:, t]
                            nc.tensor.matmul(
                                ps[:], ones[:], brhs, start=False, stop=True
                            )
                        else:
                            nc.tensor.matmul(ps[:], lhsT, rhs, start=True, stop=True)
                        nc.tensor.matmul(
                            om[:], lhsT, rhs, start=(ti == 0), stop=(ti == KK - 1)
                        )
                        if with_bias:
                            nc.tensor.matmul(
                                om[:], ones[:], brhs, start=False, stop=False,
                                skip_group_check=True,
                            )
                        if g == 0 and cc < 2:
                            # DVE is idle during the prologue: help drain the
                            # first chunks' PSUM so priors land sooner
                            nc.vector.tensor_copy(
                                out=priors[:, cc, t],
                                in_=ps[:].rearrange("p (a b) -> p a b", a=CCH),
                            )
                        else:
                            nc.scalar.copy(
                                out=priors[:, cc, t],
                                in_=ps[:].rearrange("p (a b) -> p a b", a=CCH),
                            )
                    # o2 = 2*mean = (2/9) sum_t priors  (bf16)
                    nc.scalar.activation(
                        out=o2[:, cc], in_=om[:], func=AF.Copy, scale=2.0 / KK
                    )
                    ob = (
                        o2[:, cc]
                        .rearrange("p (a b) -> p a b", a=CCH)
                        .unsqueeze(1)
                        .broadcast_to((128, KK, CCH, NCAPS))
                    )
                    nc.vector.tensor_mul(tprod0[:, cc], priors[:, cc], ob)

                # n[t,cap] = sum_cch p^2.  Group 0 has no predecessor to hide
                # the 31us ACT square under, so its square runs on the DVE
                # (which would otherwise idle in the prologue); later groups
                # use ACT, hidden under the previous group's routing.
                nsq = big_pool.tile([128, GRP, KK, CCH, NCAPS], bf16, tag="big")
                if g == 0:
                    nc.vector.tensor_mul(nsq[:], priors[:], priors[:])
                else:
                    nc.scalar.activation(out=nsq[:], in_=priors[:], func=AF.Square)
                cch_ladder_inplace(nsq[:])
                ntile = npool.tile([128, GRP, KK, NCAPS], bf16)
                nc.vector.tensor_copy(out=ntile[:], in_=nsq[:, :, :, 0])

                e2b = emit_e2(o2)
                for it in range(NIT):
                    last = it == NIT - 1
                    # s = <o2, p_t>: product + in-place cch halving ladder
                    if it == 0:
                        tprod = tprod0
                    else:
                        tprod = big_pool.tile(
                            [128, GRP, KK, CCH, NCAPS], bf16, tag="big"
                        )
                        ob = (
                            o2[:]
                            .rearrange("p c (a b) -> p c a b", a=CCH)
                            .unsqueeze(2)
                            .broadcast_to((128, GRP, KK, CCH, NCAPS))
                        )
                        nc.vector.tensor_mul(tprod[:], priors[:], ob)
                    cch_ladder_inplace(tprod[:])
                    s = tprod[:, :, :, 0]  # [128, GRP, KK, NCAPS] view

                    # dist = (n - s) + e'  (bf16, 2x)
                    dist = small_pool.tile([128, GRP, KK, NCAPS], bf16, tag="dist")
                    nc.vector.tensor_sub(dist[:], ntile[:], s)
                    nc.vector.tensor_add(
                        dist[:],
                        dist[:],
                        e2b[:].unsqueeze(2).broadcast_to((128, GRP, KK, NCAPS)),
                    )
                    # cwu = dist^-0.5 (single-table-set rsqrt on ACT)
                    cwu = small_pool.tile([128, GRP, KK, NCAPS], bf16, tag="cwu")
                    nc.scalar.activation(
                        out=cwu[:], in_=dist[:], func=AF.Abs_reciprocal_sqrt
                    )
                    # alpha = cwu / sum_t cwu (doubled except last iter);
                    # cwsum via 2x halving ladder, f32 at the last level
                    ch1 = small_pool.tile([128, GRP, 4, NCAPS], bf16, tag="ch1")
                    nc.vector.tensor_add(ch1[:], cwu[:, :, 0:4], cwu[:, :, 4:8])
                    nc.vector.tensor_add(
                        ch1[:, :, 0:2], ch1[:, :, 0:2], ch1[:, :, 2:4]
                    )
                    nc.vector.tensor_add(ch1[:, :, 0], ch1[:, :, 0], ch1[:, :, 1])
                    cwsum = small_pool.tile([128, GRP, NCAPS], f32, tag="cwsum")
                    nc.vector.tensor_add(cwsum[:], ch1[:, :, 0], cwu[:, :, 8])
                    rs = small_pool.tile([128, GRP, NCAPS], f32, tag="rs")
                    nc.vector.reciprocal_approx_fast(rs[:], cwsum[:])
                    rsb = small_pool.tile([128, GRP, NCAPS], bf16, tag="rsb")
                    nc.vector.tensor_scalar_mul(
                        rsb[:], rs[:], 1.0 if last else 2.0
                    )
                    alpha = small_pool.tile([128, GRP, KK, NCAPS], bf16, tag="al")
                    nc.vector.tensor_mul(
                        alpha[:],
                        cwu[:],
                        rsb[:].unsqueeze(2).broadcast_to((128, GRP, KK, NCAPS)),
                    )

                    # o' = sum_t alpha_t p_t: product + in-place tap halving
                    if not last:
                        wprod = big_pool.tile(
                            [128, GRP, KK, CCH, NCAPS], bf16, tag="big"
                        )
                        ab = alpha[:].unsqueeze(3).broadcast_to(
                            (128, GRP, KK, CCH, NCAPS)
                        )
                        nc.vector.tensor_mul(wprod[:], priors[:], ab)
                        wp = wprod[:].rearrange("p c t a b -> p c t (a b)")
                        nc.vector.tensor_add(
                            wp[:, :, 0:4], wp[:, :, 0:4], wp[:, :, 4:8]
                        )
                        nc.vector.tensor_add(
                            wp[:, :, 0:2], wp[:, :, 0:2], wp[:, :, 2:4]
                        )
                        nc.vector.tensor_add(wp[:, :, 0], wp[:, :, 0], wp[:, :, 1])
                        o2 = o_pool.tile([128, GRP, NC], bf16)
                        nc.vector.tensor_add(o2[:], wp[:, :, 0], wp[:, :, 8])
                        e2b = emit_e2(o2)
                        continue

                    # last iteration: per sub-chunk so the output transposes
                    # overlap the other sub-chunks' weighted sums
                    for cc in range(GRP):
                        ch = GRP * g + cc
                        wprod = big_pool.tile(
                            [128, KK, CCH, NCAPS], bf16, tag="big"
                        )
                        ab = alpha[:, cc].unsqueeze(2).broadcast_to(
                            (128, KK, CCH, NCAPS)
                        )
                        nc.vector.tensor_mul(wprod[:], priors[:, cc], ab)
                        wp = wprod[:].rearrange("p t a b -> p t (a b)")
                        nc.vector.tensor_add(wp[:, 0:4], wp[:, 0:4], wp[:, 4:8])
                        nc.vector.tensor_add(wp[:, 0:2], wp[:, 0:2], wp[:, 2:4])
                        nc.vector.tensor_add(wp[:, 0], wp[:, 0], wp[:, 1])
                        onat = o_pool.tile([128, NC], f32, tag="onat")
                        nc.vector.tensor_add(
                            onat[:].rearrange(
                                "p (cap cch) -> p cch cap", cch=CCH
                            ),
                            wp[:, 0].rearrange(
                                "p (cch cap) -> p cch cap", cch=CCH
                            ),
                            wp[:, 8].rearrange(
                                "p (cch cap) -> p cch cap", cch=CCH
                            ),
                        )
                        ot = small_pool.tile([128, 4, 128], bf16, tag="ostage")
                        for blk in range(4):
                            tp = tpp.tile([128, 128], f32)
                            nc.tensor.transpose(
                                tp[:],
                                onat[:, 128 * blk : 128 * (blk + 1)],
                                ident[:],
                            )
                            nc.scalar.copy(out=ot[:, blk], in_=tp[:])
                        # cast-DMA (SWDGE) on the idle gpsimd queue: bf16
                        # staging halves the SBUF reads; dram stays f32
                        nc.gpsimd.dma_start(
                            out=out_d[:, 128 * ch : 128 * (ch + 1)].rearrange(
                                "(blk n) q -> n blk q", blk=4
                            ),
                            in_=ot[:],
                        )
    nc.compile()
    return nc


def _get_nc(with_bias: bool):
    key = ("nc", with_bias)
    if key not in _cache:
        _cache[key] = _build(with_bias)
    return _cache[key]


def kernel(input, weight, bias, _trace=False):
    import ml_dtypes
    from concourse.bass_utils import run_bass_kernel_spmd

    bf = ml_dtypes.bfloat16
    input = np.ascontiguousarray(np.asarray(input, dtype=np.float32).astype(bf))
    # permute weight columns to (cch, cap) order and put the contraction
    # dim (c) first, so the device DMA is one contiguous read
    w = np.asarray(weight, dtype=np.float32).reshape(KK, C, NCAPS, CCH)
    w = np.ascontiguousarray(
        w.transpose(1, 0, 3, 2).reshape(C, KK, NC).astype(bf)
    )
    b = np.asarray(bias, dtype=np.float32).reshape(KK, NCAPS, CCH)
    b = np.ascontiguousarray(b.transpose(0, 2, 1).reshape(KK, NC).astype(bf))
    with_bias = bool(np.any(np.asarray(bias)))

    nc = _get_nc(with_bias)
    in_maps = [
        {"x": np.ascontiguousarray(input[i]), "w": w, "b": b} for i in range(B)
    ]
    res = run_bass_kernel_spmd(
        nc, in_maps, core_ids=list(range(B)), trace=_trace
    )
    _cache["last_result"] = res
    out = np.stack(
        [r["out"].reshape(NC, H, W) for r in res.results], axis=0
    )
    return out
